# revision 24
# baseline (speedup 1.0000x reference)
"""Distributed attention kernel for 8 TRN2 NeuronCores.

Problem: cross-attention (q from target, k/v from reference) with
B=2, N=M=2048, C=1024, H=16 heads, hd=64, followed by an output
projection with bias.

Sharding (data + head parallel):
  core c in 0..7 owns heads {2c, 2c+1} for BOTH batches. It computes
  K^T/Q^T/V for its heads and attention (softmax over keys), producing
  x_local^T [128ch, 2048m] per batch. One AllToAll PER BATCH then
  redistributes so core c owns output rows [c*256, (c+1)*256) of that
  batch with ALL 1024 channels; core c applies the full Wproj
  ([1024,1024], replicated) + bias to its row-block. The batch-0
  AllToAll and projection overlap the batch-1 attention compute.

Everything on-device runs in a transposed layout ([channels, seq]) so
no transposes of the big activations are needed: the host
pre-transposes the inputs and re-transposes each core's [1024, 256]
output blocks. Matmuls run in bf16 (f32 PSUM accumulation); softmax
denominators come free as a ones-column appended to V; reciprocal
broadcast goes through small DRAM bounce tiles (DMA can move/broadcast
across partitions; DVE cannot).
"""

import functools

import numpy as np

B = 2
N = 2048  # reference rows (keys)
M = 2048  # target rows (queries)
C = 1024
H = 16
HD = 64
NCORES = 8
HPC = 2  # heads per core
CHPC = HPC * HD  # 128 channels per core
MBLK = M // NCORES  # 256 output rows owned per core (per batch)
MT = 512  # attention m-tile
KC = N // 128  # 16 key chunks
CC = C // 128  # 8 contraction chunks
NMT = M // MT  # 4 m-tiles per batch


@functools.lru_cache(maxsize=1)
def _build():
    import concourse.bacc as bacc
    import concourse.mybir as mybir
    import concourse.tile as tile

    fp32 = mybir.dt.float32
    bf16 = mybir.dt.bfloat16
    AF = mybir.ActivationFunctionType

    nc = bacc.Bacc("TRN2", target_bir_lowering=False, debug=False, num_devices=NCORES)

    xrefT = nc.dram_tensor("xrefT", [B, C, N], bf16, kind="ExternalInput")
    xtgtT = nc.dram_tensor("xtgtT", [B, C, M], bf16, kind="ExternalInput")
    wq = nc.dram_tensor("wq", [C, CHPC], bf16, kind="ExternalInput")
    wk = nc.dram_tensor("wk", [C, CHPC], bf16, kind="ExternalInput")
    wv = nc.dram_tensor("wv", [C, CHPC], bf16, kind="ExternalInput")
    wproj = nc.dram_tensor("wproj", [C, C], bf16, kind="ExternalInput")
    bproj = nc.dram_tensor("bproj", [C], fp32, kind="ExternalInput")
    out = nc.dram_tensor("out", [B, C, MBLK], fp32, kind="ExternalOutput")

    with tile.TileContext(nc) as tc:
        with (
            tc.tile_pool(name="wpool", bufs=1) as wpool,
            tc.tile_pool(name="xpool", bufs=10) as xpool,
            tc.tile_pool(name="kqv", bufs=1) as kqv,
            tc.tile_pool(name="epool", bufs=3) as epool,
            tc.tile_pool(name="spool", bufs=4) as spool,
            tc.tile_pool(name="stage", bufs=2) as stpool,
            tc.tile_pool(name="ppool", bufs=2) as ppool,
            tc.tile_pool(name="psA", bufs=3, space="PSUM") as psA,
            tc.tile_pool(name="psO", bufs=2, space="PSUM") as psO,
            tc.tile_pool(name="dram", bufs=1, space="DRAM") as dpool,
        ):
            nrows = HPC * NMT  # 8 denominator rows per batch

            # ---- weight loads (emitted first; DMA engines run ahead) ----
            wq_sb = wpool.tile([128, CC, CHPC], bf16)
            wk_sb = wpool.tile([128, CC, CHPC], bf16)
            wv_sb = wpool.tile([128, CC, CHPC], bf16)
            for cc in range(CC):
                nc.sync.dma_start(wk_sb[:, cc, :], wk[cc * 128:(cc + 1) * 128, :])
                nc.sync.dma_start(wv_sb[:, cc, :], wv[cc * 128:(cc + 1) * 128, :])
                nc.sync.dma_start(wq_sb[:, cc, :], wq[cc * 128:(cc + 1) * 128, :])

            kT = [kqv.tile([128, N], bf16, tag=f"kT{b}", name=f"kT{b}") for b in range(B)]
            qT = [kqv.tile([128, M], bf16, tag=f"qT{b}", name=f"qT{b}") for b in range(B)]
            vA = [
                kqv.tile([128, KC, HPC, HD + 1], bf16, tag=f"vA{b}", name=f"vA{b}")
                for b in range(B)
            ]
            oU = [kqv.tile([128, M], bf16, tag=f"oU{b}", name=f"oU{b}") for b in range(B)]
            for b in range(B):
                nc.vector.memset(vA[b][:, :, :, HD:HD + 1], 1.0)

            a2a_in = [
                dpool.tile([NCORES, CHPC, MBLK], bf16, tag=f"a2a_in{b}", name=f"a2a_in{b}")
                for b in range(B)
            ]
            a2a_out = [
                dpool.tile([NCORES, CHPC, MBLK], bf16, tag=f"a2a_out{b}", name=f"a2a_out{b}")
                for b in range(B)
            ]
            ddram = dpool.tile([B, nrows, MT], fp32, tag="ddram", name="ddram")
            rdram = dpool.tile([B, nrows, MT], bf16, tag="rdram", name="rdram")

            xch = {}  # (tensor, b, cc) -> sbuf chunk tile

            def load_chunk(which, b, cc):
                src = xrefT if which == "r" else xtgtT
                t = xpool.tile([128, N], bf16, tag="x", name=f"x{which}{b}_{cc}")
                # spread across the three DMA-capable queues; keep the ACT
                # queue clear once attention (exp-bound) is running
                if b == 0:
                    eng = [nc.sync, nc.gpsimd, nc.scalar][cc % 3]
                else:
                    eng = [nc.sync, nc.gpsimd][cc % 2]
                eng.dma_start(t[:], src[b, cc * 128:(cc + 1) * 128, :])
                xch[(which, b, cc)] = t

            def kt_half(b, half, w_sb, dstT, which):
                ps = psA.tile([128, 2 * MT], fp32, tag="big", name=f"ps{which}{b}{half}")
                for nt in range(2):
                    g = half * 2 + nt
                    for cc in range(CC):
                        nc.tensor.matmul(
                            ps[:, nt * MT:(nt + 1) * MT],
                            lhsT=w_sb[:, cc, :],
                            rhs=xch[(which, b, cc)][:, g * MT:(g + 1) * MT],
                            start=(cc == 0),
                            stop=(cc == CC - 1),
                        )
                nc.vector.tensor_copy(
                    dstT[:, half * 2 * MT:(half + 1) * 2 * MT], ps[:]
                )

            def v_half(b, half):
                ps = psA.tile([128, 2 * MT], fp32, tag="big", name=f"psv{b}{half}")
                for k in range(8):
                    kc = half * 8 + k
                    for cc in range(CC):
                        nc.tensor.matmul(
                            ps[:, k * 128:(k + 1) * 128],
                            lhsT=xch[("r", b, cc)][:, kc * 128:(kc + 1) * 128],
                            rhs=wv_sb[:, cc, :],
                            start=(cc == 0),
                            stop=(cc == CC - 1),
                        )
                nc.vector.tensor_copy(
                    vA[b][:, half * 8:(half + 1) * 8, :, 0:HD],
                    ps[:].rearrange("p (k h d) -> p k h d", k=8, h=HPC),
                )

            scale = float(HD) ** -0.5

            def attn_mt(b, mt):
                po = [
                    psO.tile([HD + 1, MT], fp32, tag="o", name=f"po{h}")
                    for h in range(HPC)
                ]
                for kc in range(0, KC, 2):
                    pss = [
                        psA.tile([128, 2 * MT], fp32, tag="big", name="pss")
                        for _ in range(HPC)
                    ]
                    for j in range(2):
                        # the two heads sit at partitions 0-63 / 64-127 so the
                        # PE row-groups run their K=64 matmuls concurrently
                        for h in range(HPC):
                            nc.tensor.matmul(
                                pss[h][:, j * MT:(j + 1) * MT],
                                lhsT=kT[b][h * HD:(h + 1) * HD, (kc + j) * 128:(kc + j + 1) * 128],
                                rhs=qT[b][h * HD:(h + 1) * HD, mt * MT:(mt + 1) * MT],
                                start=True,
                                stop=True,
                            )
                    eS = [
                        epool.tile([128, 2, MT], bf16, tag="eS", name="eS")
                        for _ in range(HPC)
                    ]
                    for h in range(HPC):
                        nc.scalar.activation(
                            eS[h][:].rearrange("p a b -> p (a b)"),
                            pss[h][:],
                            AF.Exp,
                            scale=scale,
                        )
                    # stream AV immediately so the PE stays busy during the
                    # ACT-bound exp phase
                    for h in range(HPC):
                        for j in range(2):
                            nc.tensor.matmul(
                                po[h][:],
                                lhsT=vA[b][:, kc + j, h, :],
                                rhs=eS[h][:, j, :],
                                start=(kc == 0 and j == 0),
                                stop=(kc == KC - 2 and j == 1),
                            )
                for h in range(HPC):
                    nc.vector.tensor_copy(
                        oU[b][h * HD:(h + 1) * HD, mt * MT:(mt + 1) * MT],
                        po[h][0:HD, :],
                    )
                    dt = spool.tile([HD + 1, MT], fp32, tag="dt", name="dt")
                    nc.vector.tensor_copy(dt[HD:HD + 1, :], po[h][HD:HD + 1, :])
                    nc.sync.dma_start(
                        ddram[b, mt * HPC + h:mt * HPC + h + 1, :],
                        dt[HD:HD + 1, :],
                    )

            def norm_and_a2a(b):
                dNs = spool.tile([nrows, MT], fp32, tag="dNs", name=f"dNs{b}")
                nc.sync.dma_start(dNs[:], ddram[b])
                rf = spool.tile([nrows, MT], fp32, tag="rf", name=f"rf{b}")
                nc.vector.reciprocal(rf[:], dNs[:])
                rN = spool.tile([nrows, MT], bf16, tag="rN", name=f"rN{b}")
                nc.vector.tensor_copy(rN[:], rf[:])
                nc.sync.dma_start(rdram[b], rN[:])
                for mt in range(NMT):
                    xst = stpool.tile([128, MT], bf16, tag="stage")
                    rb = spool.tile([128, MT], bf16, tag="rb", name="rb")
                    for h in range(HPC):
                        r = mt * HPC + h
                        nc.sync.dma_start(
                            rb[h * HD:(h + 1) * HD, :],
                            rdram[b, r:r + 1, :].to_broadcast((HD, MT)),
                        )
                    nc.vector.tensor_mul(
                        xst[:], oU[b][:, mt * MT:(mt + 1) * MT], rb[:]
                    )
                    for s in range(MT // MBLK):
                        nc.sync.dma_start(
                            a2a_in[b][mt * (MT // MBLK) + s],
                            xst[:, s * MBLK:(s + 1) * MBLK],
                        )
                nc.gpsimd.collective_compute(
                    "AllToAll",
                    mybir.AluOpType.bypass,
                    replica_groups=[list(range(NCORES))],
                    ins=[a2a_in[b][:].opt()],
                    outs=[a2a_out[b][:].opt()],
                )

            def proj(b):
                y_sb = ppool.tile([128, NCORES, MBLK], bf16, tag="y", name=f"y{b}")
                for i in range(NCORES):
                    nc.sync.dma_start(y_sb[:, i, :], a2a_out[b][i])
                for oc in range(CC):
                    psb = psA.tile([128, 2 * MT], fp32, tag="big", name="pp")
                    ps = psb[:, 0:MBLK]
                    for cc in range(CC):
                        nc.tensor.matmul(
                            ps[:],
                            lhsT=wp_sb[:, cc, oc * 128:(oc + 1) * 128],
                            rhs=y_sb[:, cc, :],
                            start=(cc == 0),
                            stop=(cc == CC - 1),
                        )
                    osb = ppool.tile([128, MBLK], fp32, tag="outsb", name="osb")
                    nc.scalar.activation(
                        osb[:], ps[:], AF.Identity, bias=bias_sb[:, oc:oc + 1]
                    )
                    nc.sync.dma_start(out[b, oc * 128:(oc + 1) * 128, :], osb[:])

            # ================= emission schedule =================
            # batch-0 QKV, chunk-streamed
            for cc in range(CC):
                load_chunk("r", 0, cc)
            for cc in range(CC):
                load_chunk("t", 0, cc)
            kt_half(0, 0, wk_sb, kT[0], "r")
            v_half(0, 0)
            kt_half(0, 1, wk_sb, kT[0], "r")
            v_half(0, 1)
            kt_half(0, 0, wq_sb, qT[0], "t")

            # batch-1 chunk loads queued early (DMA runs ahead of compute)
            for cc in range(CC):
                load_chunk("r", 1, cc)
            for cc in range(CC):
                load_chunk("t", 1, cc)

            # wproj/bias loads (needed by proj(0) mid-way through)
            wp_sb = wpool.tile([128, CC, C], bf16, name="wp_sb")
            for cc in range(CC):
                nc.sync.dma_start(wp_sb[:, cc, :], wproj[cc * 128:(cc + 1) * 128, :])
            bias_sb = wpool.tile([128, CC], fp32, name="bias_sb")
            nc.sync.dma_start(bias_sb[:], bproj.ap().rearrange("(a p) -> p a", p=128))

            # batch-1 QKV pieces interleaved between batch-0 attention m-tiles
            qkv1 = [
                lambda: kt_half(1, 0, wk_sb, kT[1], "r"),
                lambda: v_half(1, 0),
                lambda: (kt_half(1, 1, wk_sb, kT[1], "r"), v_half(1, 1)),
                lambda: (
                    kt_half(1, 0, wq_sb, qT[1], "t"),
                    kt_half(1, 1, wq_sb, qT[1], "t"),
                ),
            ]
            attn_mt(0, 0)
            kt_half(0, 1, wq_sb, qT[0], "t")
            attn_mt(0, 1)
            qkv1[0]()
            qkv1[1]()
            attn_mt(0, 2)
            qkv1[2]()
            attn_mt(0, 3)
            qkv1[3]()

            # batch-0 normalize + AllToAll fire while batch-1 attention runs
            norm_and_a2a(0)
            attn_mt(1, 0)
            attn_mt(1, 1)
            attn_mt(1, 2)
            proj(0)  # a2a(0) long done; PE work overlaps the tail AllToAll
            attn_mt(1, 3)
            norm_and_a2a(1)
            proj(1)

    nc.compile()
    return nc


def _shard_inputs(reference_data, target_data, Wq, Wkv, Wproj, bproj):
    import ml_dtypes

    bf16 = ml_dtypes.bfloat16
    xrefT = np.ascontiguousarray(
        np.asarray(reference_data, dtype=np.float32).transpose(0, 2, 1)
    ).astype(bf16)
    xtgtT = np.ascontiguousarray(
        np.asarray(target_data, dtype=np.float32).transpose(0, 2, 1)
    ).astype(bf16)
    Wq = np.asarray(Wq, dtype=np.float32)
    Wkv = np.asarray(Wkv, dtype=np.float32)
    Wproj_b = np.asarray(Wproj, dtype=np.float32).astype(bf16)
    bproj = np.asarray(bproj, dtype=np.float32)

    in_maps = []
    for c in range(NCORES):
        lo, hi = c * CHPC, (c + 1) * CHPC
        in_maps.append(
            {
                "xrefT": xrefT,
                "xtgtT": xtgtT,
                "wq": Wq[:, lo:hi].astype(bf16),
                "wk": Wkv[:, lo:hi].astype(bf16),
                "wv": Wkv[:, C + lo:C + hi].astype(bf16),
                "wproj": Wproj_b,
                "bproj": bproj,
            }
        )
    return in_maps


def _ensure_ntff_hook():
    """Register the axon NTFF profile hook if the image's antenv lacks it."""
    try:
        import antenv.axon_hooks  # noqa: F401

        return
    except ImportError:
        pass
    import sys
    import types

    import antenv

    mod = types.ModuleType("antenv.axon_hooks")
    state = {"hook": None}
    mod.set_axon_ntff_profile_hook = lambda h: state.__setitem__("hook", h)
    mod.get_axon_ntff_profile_hook = lambda: state["hook"]
    sys.modules["antenv.axon_hooks"] = mod
    antenv.axon_hooks = mod
    try:
        from trn_agent_boot.trn_boot import _ntff_profile_via_ctypes

        mod.set_axon_ntff_profile_hook(
            _ntff_profile_via_ctypes("/opt/axon/libaxon_pjrt.so")
        )
    except Exception:
        pass


def run(inputs: dict, trace: bool = False):
    """Compile (cached), run on 8 cores, return (full_output, BassKernelResults)."""
    from concourse.bass_utils import run_bass_kernel_spmd

    if trace:
        _ensure_ntff_hook()
    nc = _build()
    in_maps = _shard_inputs(**inputs)
    res = run_bass_kernel_spmd(
        nc, in_maps, core_ids=list(range(NCORES)), trace=trace
    )
    full = np.zeros((B, M, C), dtype=np.float32)
    for c in range(NCORES):
        blk = np.asarray(res.results[c]["out"], dtype=np.float32)  # [B, C, MBLK]
        for b in range(B):
            full[b, c * MBLK:(c + 1) * MBLK, :] = blk[b].T
    return full, res


def kernel(reference_data, target_data, Wq, Wkv, Wproj, bproj) -> np.ndarray:
    full, _ = run(
        {
            "reference_data": reference_data,
            "target_data": target_data,
            "Wq": Wq,
            "Wkv": Wkv,
            "Wproj": Wproj,
            "bproj": bproj,
        }
    )
    return full


# revision 25
# speedup vs baseline: 1.0105x; 1.0105x over previous
"""Distributed attention kernel for 8 TRN2 NeuronCores.

Problem: cross-attention (q from target, k/v from reference) with
B=2, N=M=2048, C=1024, H=16 heads, hd=64, followed by an output
projection with bias.

Sharding (data + head parallel):
  core c in 0..7 owns heads {2c, 2c+1} for BOTH batches. It computes
  K^T/Q^T/V for its heads and attention (softmax over keys), producing
  x_local^T [128ch, 2048m] per batch. One AllToAll PER BATCH then
  redistributes so core c owns output rows [c*256, (c+1)*256) of that
  batch with ALL 1024 channels; core c applies the full Wproj
  ([1024,1024], replicated) + bias to its row-block. The batch-0
  AllToAll and projection overlap the batch-1 attention compute.

Everything on-device runs in a transposed layout ([channels, seq]) so
no transposes of the big activations are needed: the host
pre-transposes the inputs and re-transposes each core's [1024, 256]
output blocks. Matmuls run in bf16 (f32 PSUM accumulation); softmax
denominators come free as a ones-column appended to V; reciprocal
broadcast goes through small DRAM bounce tiles (DMA can move/broadcast
across partitions; DVE cannot).
"""

import functools

import numpy as np

B = 2
N = 2048  # reference rows (keys)
M = 2048  # target rows (queries)
C = 1024
H = 16
HD = 64
NCORES = 8
HPC = 2  # heads per core
CHPC = HPC * HD  # 128 channels per core
MBLK = M // NCORES  # 256 output rows owned per core (per batch)
MT = 512  # attention m-tile
KC = N // 128  # 16 key chunks
CC = C // 128  # 8 contraction chunks
NMT = M // MT  # 4 m-tiles per batch


@functools.lru_cache(maxsize=1)
def _build():
    import concourse.bacc as bacc
    import concourse.mybir as mybir
    import concourse.tile as tile

    fp32 = mybir.dt.float32
    bf16 = mybir.dt.bfloat16
    AF = mybir.ActivationFunctionType

    nc = bacc.Bacc("TRN2", target_bir_lowering=False, debug=False, num_devices=NCORES)

    xrefT = nc.dram_tensor("xrefT", [B, C, N], bf16, kind="ExternalInput")
    xtgtT = nc.dram_tensor("xtgtT", [B, C, M], bf16, kind="ExternalInput")
    wq = nc.dram_tensor("wq", [C, CHPC], bf16, kind="ExternalInput")
    wk = nc.dram_tensor("wk", [C, CHPC], bf16, kind="ExternalInput")
    wv = nc.dram_tensor("wv", [C, CHPC], bf16, kind="ExternalInput")
    wproj = nc.dram_tensor("wproj", [C, C], bf16, kind="ExternalInput")
    bproj = nc.dram_tensor("bproj", [C], fp32, kind="ExternalInput")
    out = nc.dram_tensor("out", [B, C, MBLK], fp32, kind="ExternalOutput")

    with tile.TileContext(nc) as tc:
        with (
            tc.tile_pool(name="wpool", bufs=1) as wpool,
            tc.tile_pool(name="xpool", bufs=10) as xpool,
            tc.tile_pool(name="kqv", bufs=1) as kqv,
            tc.tile_pool(name="epool", bufs=3) as epool,
            tc.tile_pool(name="spool", bufs=4) as spool,
            tc.tile_pool(name="stage", bufs=2) as stpool,
            tc.tile_pool(name="ppool", bufs=2) as ppool,
            tc.tile_pool(name="psA", bufs=3, space="PSUM") as psA,
            tc.tile_pool(name="psO", bufs=2, space="PSUM") as psO,
            tc.tile_pool(name="dram", bufs=1, space="DRAM") as dpool,
        ):
            nrows = HPC * NMT  # 8 denominator rows per batch

            # ---- weight loads (emitted first; DMA engines run ahead) ----
            wq_sb = wpool.tile([128, CC, CHPC], bf16)
            wk_sb = wpool.tile([128, CC, CHPC], bf16)
            wv_sb = wpool.tile([128, CC, CHPC], bf16)
            for cc in range(CC):
                nc.sync.dma_start(wk_sb[:, cc, :], wk[cc * 128:(cc + 1) * 128, :])
                nc.sync.dma_start(wv_sb[:, cc, :], wv[cc * 128:(cc + 1) * 128, :])
                nc.sync.dma_start(wq_sb[:, cc, :], wq[cc * 128:(cc + 1) * 128, :])

            kT = [kqv.tile([128, N], bf16, tag=f"kT{b}", name=f"kT{b}") for b in range(B)]
            qT = [kqv.tile([128, M], bf16, tag=f"qT{b}", name=f"qT{b}") for b in range(B)]
            vA = [
                kqv.tile([128, KC, HPC, HD + 1], bf16, tag=f"vA{b}", name=f"vA{b}")
                for b in range(B)
            ]
            oU = [kqv.tile([128, M], bf16, tag=f"oU{b}", name=f"oU{b}") for b in range(B)]
            for b in range(B):
                nc.vector.memset(vA[b][:, :, :, HD:HD + 1], 1.0)

            a2a_in = [
                dpool.tile([NCORES, CHPC, MBLK], bf16, tag=f"a2a_in{b}", name=f"a2a_in{b}")
                for b in range(B)
            ]
            a2a_out = [
                dpool.tile([NCORES, CHPC, MBLK], bf16, tag=f"a2a_out{b}", name=f"a2a_out{b}")
                for b in range(B)
            ]
            ddram = dpool.tile([B, nrows, MT], fp32, tag="ddram", name="ddram")
            rdram = dpool.tile([B, nrows, MT], bf16, tag="rdram", name="rdram")

            xch = {}  # (tensor, b, cc) -> sbuf chunk tile

            def load_chunk(which, b, cc):
                src = xrefT if which == "r" else xtgtT
                t = xpool.tile([128, N], bf16, tag="x", name=f"x{which}{b}_{cc}")
                # spread across the three DMA-capable queues; keep the ACT
                # queue clear once attention (exp-bound) is running
                if b == 0:
                    eng = [nc.sync, nc.gpsimd, nc.scalar][cc % 3]
                else:
                    eng = [nc.sync, nc.gpsimd][cc % 2]
                eng.dma_start(t[:], src[b, cc * 128:(cc + 1) * 128, :])
                xch[(which, b, cc)] = t

            def kt_half(b, half, w_sb, dstT, which):
                ps = psA.tile([128, 2 * MT], fp32, tag="big", name=f"ps{which}{b}{half}")
                for nt in range(2):
                    g = half * 2 + nt
                    for cc in range(CC):
                        nc.tensor.matmul(
                            ps[:, nt * MT:(nt + 1) * MT],
                            lhsT=w_sb[:, cc, :],
                            rhs=xch[(which, b, cc)][:, g * MT:(g + 1) * MT],
                            start=(cc == 0),
                            stop=(cc == CC - 1),
                        )
                nc.vector.tensor_copy(
                    dstT[:, half * 2 * MT:(half + 1) * 2 * MT], ps[:]
                )

            def v_half(b, half):
                ps = psA.tile([128, 2 * MT], fp32, tag="big", name=f"psv{b}{half}")
                for k in range(8):
                    kc = half * 8 + k
                    for cc in range(CC):
                        nc.tensor.matmul(
                            ps[:, k * 128:(k + 1) * 128],
                            lhsT=xch[("r", b, cc)][:, kc * 128:(kc + 1) * 128],
                            rhs=wv_sb[:, cc, :],
                            start=(cc == 0),
                            stop=(cc == CC - 1),
                        )
                nc.vector.tensor_copy(
                    vA[b][:, half * 8:(half + 1) * 8, :, 0:HD],
                    ps[:].rearrange("p (k h d) -> p k h d", k=8, h=HPC),
                )

            scale = float(HD) ** -0.5

            def attn_mt(b, mt):
                po = [
                    psO.tile([HD + 1, MT], fp32, tag="o", name=f"po{h}")
                    for h in range(HPC)
                ]
                for kc in range(0, KC, 2):
                    pss = [
                        psA.tile([128, 2 * MT], fp32, tag="big", name="pss")
                        for _ in range(HPC)
                    ]
                    for j in range(2):
                        # the two heads sit at partitions 0-63 / 64-127 so the
                        # PE row-groups run their K=64 matmuls concurrently
                        for h in range(HPC):
                            nc.tensor.matmul(
                                pss[h][:, j * MT:(j + 1) * MT],
                                lhsT=kT[b][h * HD:(h + 1) * HD, (kc + j) * 128:(kc + j + 1) * 128],
                                rhs=qT[b][h * HD:(h + 1) * HD, mt * MT:(mt + 1) * MT],
                                start=True,
                                stop=True,
                            )
                    eS = [
                        epool.tile([128, 2, MT], bf16, tag="eS", name="eS")
                        for _ in range(HPC)
                    ]
                    for h in range(HPC):
                        nc.scalar.activation(
                            eS[h][:].rearrange("p a b -> p (a b)"),
                            pss[h][:],
                            AF.Exp,
                            scale=scale,
                        )
                    # stream AV immediately so the PE stays busy during the
                    # ACT-bound exp phase
                    for h in range(HPC):
                        for j in range(2):
                            nc.tensor.matmul(
                                po[h][:],
                                lhsT=vA[b][:, kc + j, h, :],
                                rhs=eS[h][:, j, :],
                                start=(kc == 0 and j == 0),
                                stop=(kc == KC - 2 and j == 1),
                            )
                for h in range(HPC):
                    nc.vector.tensor_copy(
                        oU[b][h * HD:(h + 1) * HD, mt * MT:(mt + 1) * MT],
                        po[h][0:HD, :],
                    )
                    dt = spool.tile([HD + 1, MT], fp32, tag="dt", name="dt")
                    nc.vector.tensor_copy(dt[HD:HD + 1, :], po[h][HD:HD + 1, :])
                    nc.sync.dma_start(
                        ddram[b, mt * HPC + h:mt * HPC + h + 1, :],
                        dt[HD:HD + 1, :],
                    )

            def norm_and_a2a(b):
                dNs = spool.tile([nrows, MT], fp32, tag="dNs", name=f"dNs{b}")
                nc.sync.dma_start(dNs[:], ddram[b])
                rf = spool.tile([nrows, MT], fp32, tag="rf", name=f"rf{b}")
                nc.vector.reciprocal(rf[:], dNs[:])
                rN = spool.tile([nrows, MT], bf16, tag="rN", name=f"rN{b}")
                nc.vector.tensor_copy(rN[:], rf[:])
                nc.sync.dma_start(rdram[b], rN[:])
                for mt in range(NMT):
                    xst = stpool.tile([128, MT], bf16, tag="stage")
                    rb = spool.tile([128, MT], bf16, tag="rb", name="rb")
                    for h in range(HPC):
                        r = mt * HPC + h
                        nc.sync.dma_start(
                            rb[h * HD:(h + 1) * HD, :],
                            rdram[b, r:r + 1, :].to_broadcast((HD, MT)),
                        )
                    nc.vector.tensor_mul(
                        xst[:], oU[b][:, mt * MT:(mt + 1) * MT], rb[:]
                    )
                    for s in range(MT // MBLK):
                        nc.sync.dma_start(
                            a2a_in[b][mt * (MT // MBLK) + s],
                            xst[:, s * MBLK:(s + 1) * MBLK],
                        )
                nc.gpsimd.collective_compute(
                    "AllToAll",
                    mybir.AluOpType.bypass,
                    replica_groups=[list(range(NCORES))],
                    ins=[a2a_in[b][:].opt()],
                    outs=[a2a_out[b][:].opt()],
                )

            def proj(b):
                y_sb = ppool.tile([128, NCORES, MBLK], bf16, tag="y", name=f"y{b}")
                for i in range(NCORES):
                    nc.sync.dma_start(y_sb[:, i, :], a2a_out[b][i])
                for oc in range(CC):
                    psb = psA.tile([128, 2 * MT], fp32, tag="big", name="pp")
                    ps = psb[:, 0:MBLK]
                    for cc in range(CC):
                        nc.tensor.matmul(
                            ps[:],
                            lhsT=wp_sb[:, cc, oc * 128:(oc + 1) * 128],
                            rhs=y_sb[:, cc, :],
                            start=(cc == 0),
                            stop=(cc == CC - 1),
                        )
                    osb = ppool.tile([128, MBLK], fp32, tag="outsb", name="osb")
                    nc.scalar.activation(
                        osb[:], ps[:], AF.Identity, bias=bias_sb[:, oc:oc + 1]
                    )
                    nc.sync.dma_start(out[b, oc * 128:(oc + 1) * 128, :], osb[:])

            # ================= emission schedule =================
            # batch-0 QKV, chunk-streamed
            for cc in range(CC):
                load_chunk("r", 0, cc)
            for cc in range(CC):
                load_chunk("t", 0, cc)
            kt_half(0, 0, wk_sb, kT[0], "r")
            v_half(0, 0)
            kt_half(0, 1, wk_sb, kT[0], "r")
            v_half(0, 1)
            kt_half(0, 0, wq_sb, qT[0], "t")
            kt_half(0, 1, wq_sb, qT[0], "t")

            # batch-1 chunk loads queued early (DMA runs ahead of compute)
            for cc in range(CC):
                load_chunk("r", 1, cc)
            for cc in range(CC):
                load_chunk("t", 1, cc)

            # wproj/bias loads (needed by proj(0) mid-way through)
            wp_sb = wpool.tile([128, CC, C], bf16, name="wp_sb")
            for cc in range(CC):
                nc.sync.dma_start(wp_sb[:, cc, :], wproj[cc * 128:(cc + 1) * 128, :])
            bias_sb = wpool.tile([128, CC], fp32, name="bias_sb")
            nc.sync.dma_start(bias_sb[:], bproj.ap().rearrange("(a p) -> p a", p=128))

            # batch-1 QKV pieces interleaved between batch-0 attention m-tiles
            qkv1 = [
                lambda: kt_half(1, 0, wk_sb, kT[1], "r"),
                lambda: v_half(1, 0),
                lambda: (kt_half(1, 1, wk_sb, kT[1], "r"), v_half(1, 1)),
                lambda: (
                    kt_half(1, 0, wq_sb, qT[1], "t"),
                    kt_half(1, 1, wq_sb, qT[1], "t"),
                ),
            ]
            for mt in range(NMT):
                attn_mt(0, mt)
                qkv1[mt]()

            # batch-0 normalize + AllToAll fire while batch-1 attention runs
            norm_and_a2a(0)
            attn_mt(1, 0)
            attn_mt(1, 1)
            attn_mt(1, 2)
            proj(0)  # a2a(0) long done; PE work overlaps the tail AllToAll
            attn_mt(1, 3)
            norm_and_a2a(1)
            proj(1)

    nc.compile()
    return nc


def _shard_inputs(reference_data, target_data, Wq, Wkv, Wproj, bproj):
    import ml_dtypes

    bf16 = ml_dtypes.bfloat16
    xrefT = np.ascontiguousarray(
        np.asarray(reference_data, dtype=np.float32).transpose(0, 2, 1)
    ).astype(bf16)
    xtgtT = np.ascontiguousarray(
        np.asarray(target_data, dtype=np.float32).transpose(0, 2, 1)
    ).astype(bf16)
    Wq = np.asarray(Wq, dtype=np.float32)
    Wkv = np.asarray(Wkv, dtype=np.float32)
    Wproj_b = np.asarray(Wproj, dtype=np.float32).astype(bf16)
    bproj = np.asarray(bproj, dtype=np.float32)

    in_maps = []
    for c in range(NCORES):
        lo, hi = c * CHPC, (c + 1) * CHPC
        in_maps.append(
            {
                "xrefT": xrefT,
                "xtgtT": xtgtT,
                "wq": Wq[:, lo:hi].astype(bf16),
                "wk": Wkv[:, lo:hi].astype(bf16),
                "wv": Wkv[:, C + lo:C + hi].astype(bf16),
                "wproj": Wproj_b,
                "bproj": bproj,
            }
        )
    return in_maps


def _ensure_ntff_hook():
    """Register the axon NTFF profile hook if the image's antenv lacks it."""
    try:
        import antenv.axon_hooks  # noqa: F401

        return
    except ImportError:
        pass
    import sys
    import types

    import antenv

    mod = types.ModuleType("antenv.axon_hooks")
    state = {"hook": None}
    mod.set_axon_ntff_profile_hook = lambda h: state.__setitem__("hook", h)
    mod.get_axon_ntff_profile_hook = lambda: state["hook"]
    sys.modules["antenv.axon_hooks"] = mod
    antenv.axon_hooks = mod
    try:
        from trn_agent_boot.trn_boot import _ntff_profile_via_ctypes

        mod.set_axon_ntff_profile_hook(
            _ntff_profile_via_ctypes("/opt/axon/libaxon_pjrt.so")
        )
    except Exception:
        pass


def run(inputs: dict, trace: bool = False):
    """Compile (cached), run on 8 cores, return (full_output, BassKernelResults)."""
    from concourse.bass_utils import run_bass_kernel_spmd

    if trace:
        _ensure_ntff_hook()
    nc = _build()
    in_maps = _shard_inputs(**inputs)
    res = run_bass_kernel_spmd(
        nc, in_maps, core_ids=list(range(NCORES)), trace=trace
    )
    full = np.zeros((B, M, C), dtype=np.float32)
    for c in range(NCORES):
        blk = np.asarray(res.results[c]["out"], dtype=np.float32)  # [B, C, MBLK]
        for b in range(B):
            full[b, c * MBLK:(c + 1) * MBLK, :] = blk[b].T
    return full, res


def kernel(reference_data, target_data, Wq, Wkv, Wproj, bproj) -> np.ndarray:
    full, _ = run(
        {
            "reference_data": reference_data,
            "target_data": target_data,
            "Wq": Wq,
            "Wkv": Wkv,
            "Wproj": Wproj,
            "bproj": bproj,
        }
    )
    return full


# revision 26
# speedup vs baseline: 1.1040x; 1.0925x over previous
"""Distributed attention kernel for 8 TRN2 NeuronCores.

Problem: cross-attention (q from target, k/v from reference) with
B=2, N=M=2048, C=1024, H=16 heads, hd=64, followed by an output
projection with bias.

Sharding (data + head parallel):
  core c in 0..7 owns heads {2c, 2c+1} for BOTH batches. It computes
  K^T/Q^T/V for its heads and attention (softmax over keys), producing
  x_local^T [128ch, 2048m] per batch. One AllToAll PER BATCH then
  redistributes so core c owns output rows [c*256, (c+1)*256) of that
  batch with ALL 1024 channels; core c applies the full Wproj
  ([1024,1024], replicated) + bias to its row-block. The batch-0
  AllToAll and projection overlap the batch-1 attention compute.

Everything on-device runs in a transposed layout ([channels, seq]) so
no transposes of the big activations are needed: the host
pre-transposes the inputs and re-transposes each core's [1024, 256]
output blocks. Matmuls run in bf16 (f32 PSUM accumulation); softmax
denominators come free as a ones-column appended to V; reciprocal
broadcast goes through small DRAM bounce tiles (DMA can move/broadcast
across partitions; DVE cannot).
"""

import functools

import numpy as np

B = 2
N = 2048  # reference rows (keys)
M = 2048  # target rows (queries)
C = 1024
H = 16
HD = 64
NCORES = 8
HPC = 2  # heads per core
CHPC = HPC * HD  # 128 channels per core
MBLK = M // NCORES  # 256 output rows owned per core (per batch)
MT = 512  # attention m-tile
KC = N // 128  # 16 key chunks
CC = C // 128  # 8 contraction chunks
NMT = M // MT  # 4 m-tiles per batch


@functools.lru_cache(maxsize=1)
def _build():
    import concourse.bacc as bacc
    import concourse.mybir as mybir
    import concourse.tile as tile

    fp32 = mybir.dt.float32
    bf16 = mybir.dt.bfloat16
    AF = mybir.ActivationFunctionType

    nc = bacc.Bacc("TRN2", target_bir_lowering=False, debug=False, num_devices=NCORES)

    xrefT = nc.dram_tensor("xrefT", [B, C, N], bf16, kind="ExternalInput")
    xtgtT = nc.dram_tensor("xtgtT", [B, C, M], bf16, kind="ExternalInput")
    wq = nc.dram_tensor("wq", [C, CHPC], bf16, kind="ExternalInput")
    wk = nc.dram_tensor("wk", [C, CHPC], bf16, kind="ExternalInput")
    wv = nc.dram_tensor("wv", [C, CHPC], bf16, kind="ExternalInput")
    wproj = nc.dram_tensor("wproj", [C, C], bf16, kind="ExternalInput")
    bproj = nc.dram_tensor("bproj", [C], fp32, kind="ExternalInput")
    out = nc.dram_tensor("out", [B, C, MBLK], fp32, kind="ExternalOutput")

    with tile.TileContext(nc) as tc:
        with (
            tc.tile_pool(name="wpool", bufs=1) as wpool,
            tc.tile_pool(name="xpool", bufs=10) as xpool,
            tc.tile_pool(name="kqv", bufs=1) as kqv,
            tc.tile_pool(name="epool", bufs=3) as epool,
            tc.tile_pool(name="spool", bufs=4) as spool,
            tc.tile_pool(name="stage", bufs=2) as stpool,
            tc.tile_pool(name="ppool", bufs=2) as ppool,
            tc.tile_pool(name="psA", bufs=3, space="PSUM") as psA,
            tc.tile_pool(name="psO", bufs=2, space="PSUM") as psO,
            tc.tile_pool(name="dram", bufs=1, space="DRAM") as dpool,
        ):
            nrows = HPC * NMT  # 8 denominator rows per batch

            # ---- weight loads (emitted first; DMA engines run ahead) ----
            wq_sb = wpool.tile([128, CC, CHPC], bf16)
            wk_sb = wpool.tile([128, CC, CHPC], bf16)
            wv_sb = wpool.tile([128, CC, CHPC], bf16)
            for cc in range(CC):
                nc.sync.dma_start(wk_sb[:, cc, :], wk[cc * 128:(cc + 1) * 128, :])
                nc.sync.dma_start(wv_sb[:, cc, :], wv[cc * 128:(cc + 1) * 128, :])
                nc.sync.dma_start(wq_sb[:, cc, :], wq[cc * 128:(cc + 1) * 128, :])

            kT = [kqv.tile([128, N], bf16, tag=f"kT{b}", name=f"kT{b}") for b in range(B)]
            qT = [kqv.tile([128, M], bf16, tag=f"qT{b}", name=f"qT{b}") for b in range(B)]
            vA = [
                kqv.tile([128, KC, HPC, HD + 1], bf16, tag=f"vA{b}", name=f"vA{b}")
                for b in range(B)
            ]
            oU = [kqv.tile([128, M], bf16, tag=f"oU{b}", name=f"oU{b}") for b in range(B)]
            for b in range(B):
                nc.vector.memset(vA[b][:, :, :, HD:HD + 1], 1.0)

            a2a_in = [
                dpool.tile([NCORES, CHPC, MBLK], bf16, tag=f"a2a_in{b}", name=f"a2a_in{b}")
                for b in range(B)
            ]
            a2a_out = [
                dpool.tile([NCORES, CHPC, MBLK], bf16, tag=f"a2a_out{b}", name=f"a2a_out{b}")
                for b in range(B)
            ]
            ddram = dpool.tile([B, nrows, MT], fp32, tag="ddram", name="ddram")
            rdram = dpool.tile([B, nrows, MT], bf16, tag="rdram", name="rdram")

            xch = {}  # (tensor, b, cc) -> sbuf chunk tile

            def load_chunk(which, b, cc):
                src = xrefT if which == "r" else xtgtT
                t = xpool.tile([128, N], bf16, tag="x", name=f"x{which}{b}_{cc}")
                # spread across the three DMA-capable queues; keep the ACT
                # queue clear once attention (exp-bound) is running
                if b == 0:
                    eng = [nc.sync, nc.gpsimd, nc.scalar][cc % 3]
                else:
                    eng = [nc.sync, nc.gpsimd][cc % 2]
                eng.dma_start(t[:], src[b, cc * 128:(cc + 1) * 128, :])
                xch[(which, b, cc)] = t

            def kt_half(b, half, w_sb, dstT, which):
                ps = psA.tile([128, 2 * MT], fp32, tag="big", name=f"ps{which}{b}{half}")
                for nt in range(2):
                    g = half * 2 + nt
                    for cc in range(CC):
                        nc.tensor.matmul(
                            ps[:, nt * MT:(nt + 1) * MT],
                            lhsT=w_sb[:, cc, :],
                            rhs=xch[(which, b, cc)][:, g * MT:(g + 1) * MT],
                            start=(cc == 0),
                            stop=(cc == CC - 1),
                        )
                nc.vector.tensor_copy(
                    dstT[:, half * 2 * MT:(half + 1) * 2 * MT], ps[:]
                )

            def v_half(b, half):
                ps = psA.tile([128, 2 * MT], fp32, tag="big", name=f"psv{b}{half}")
                for k in range(8):
                    kc = half * 8 + k
                    for cc in range(CC):
                        nc.tensor.matmul(
                            ps[:, k * 128:(k + 1) * 128],
                            lhsT=xch[("r", b, cc)][:, kc * 128:(kc + 1) * 128],
                            rhs=wv_sb[:, cc, :],
                            start=(cc == 0),
                            stop=(cc == CC - 1),
                        )
                nc.vector.tensor_copy(
                    vA[b][:, half * 8:(half + 1) * 8, :, 0:HD],
                    ps[:].rearrange("p (k h d) -> p k h d", k=8, h=HPC),
                )

            scale = float(HD) ** -0.5

            def attn_mt(b, mt):
                po = [
                    psO.tile([HD + 1, MT], fp32, tag="o", name=f"po{h}")
                    for h in range(HPC)
                ]

                def av_pair(kc, eS):
                    for h in range(HPC):
                        for j in range(2):
                            nc.tensor.matmul(
                                po[h][:],
                                lhsT=vA[b][:, kc + j, h, :],
                                rhs=eS[h][:, j, :],
                                start=(kc == 0 and j == 0),
                                stop=(kc == KC - 2 and j == 1),
                            )

                # software-pipelined by one kc-pair: the AV of pair k is
                # emitted AFTER the S^T of pair k+1, so the PE always has
                # wait-free work while the ACT engine streams exps, and exps
                # run back-to-back (ACT is the kernel's bottleneck engine).
                prev = None
                for kc in range(0, KC, 2):
                    pss = [
                        psA.tile([128, 2 * MT], fp32, tag="big", name="pss")
                        for _ in range(HPC)
                    ]
                    for j in range(2):
                        # the two heads sit at partitions 0-63 / 64-127 so the
                        # PE row-groups run their K=64 matmuls concurrently
                        for h in range(HPC):
                            nc.tensor.matmul(
                                pss[h][:, j * MT:(j + 1) * MT],
                                lhsT=kT[b][h * HD:(h + 1) * HD, (kc + j) * 128:(kc + j + 1) * 128],
                                rhs=qT[b][h * HD:(h + 1) * HD, mt * MT:(mt + 1) * MT],
                                start=True,
                                stop=True,
                            )
                    eS = [
                        epool.tile([128, 2, MT], bf16, tag="eS", name="eS")
                        for _ in range(HPC)
                    ]
                    for h in range(HPC):
                        nc.scalar.activation(
                            eS[h][:].rearrange("p a b -> p (a b)"),
                            pss[h][:],
                            AF.Exp,
                            scale=scale,
                        )
                    if prev is not None:
                        av_pair(*prev)
                    prev = (kc, eS)
                av_pair(*prev)
                for h in range(HPC):
                    nc.vector.tensor_copy(
                        oU[b][h * HD:(h + 1) * HD, mt * MT:(mt + 1) * MT],
                        po[h][0:HD, :],
                    )
                    dt = spool.tile([HD + 1, MT], fp32, tag="dt", name="dt")
                    nc.vector.tensor_copy(dt[HD:HD + 1, :], po[h][HD:HD + 1, :])
                    nc.sync.dma_start(
                        ddram[b, mt * HPC + h:mt * HPC + h + 1, :],
                        dt[HD:HD + 1, :],
                    )

            def norm_and_a2a(b):
                dNs = spool.tile([nrows, MT], fp32, tag="dNs", name=f"dNs{b}")
                nc.sync.dma_start(dNs[:], ddram[b])
                rf = spool.tile([nrows, MT], fp32, tag="rf", name=f"rf{b}")
                nc.vector.reciprocal(rf[:], dNs[:])
                rN = spool.tile([nrows, MT], bf16, tag="rN", name=f"rN{b}")
                nc.vector.tensor_copy(rN[:], rf[:])
                nc.sync.dma_start(rdram[b], rN[:])
                for mt in range(NMT):
                    xst = stpool.tile([128, MT], bf16, tag="stage")
                    rb = spool.tile([128, MT], bf16, tag="rb", name="rb")
                    for h in range(HPC):
                        r = mt * HPC + h
                        nc.sync.dma_start(
                            rb[h * HD:(h + 1) * HD, :],
                            rdram[b, r:r + 1, :].to_broadcast((HD, MT)),
                        )
                    nc.vector.tensor_mul(
                        xst[:], oU[b][:, mt * MT:(mt + 1) * MT], rb[:]
                    )
                    for s in range(MT // MBLK):
                        nc.sync.dma_start(
                            a2a_in[b][mt * (MT // MBLK) + s],
                            xst[:, s * MBLK:(s + 1) * MBLK],
                        )
                nc.gpsimd.collective_compute(
                    "AllToAll",
                    mybir.AluOpType.bypass,
                    replica_groups=[list(range(NCORES))],
                    ins=[a2a_in[b][:].opt()],
                    outs=[a2a_out[b][:].opt()],
                )

            def proj(b):
                y_sb = ppool.tile([128, NCORES, MBLK], bf16, tag="y", name=f"y{b}")
                for i in range(NCORES):
                    nc.sync.dma_start(y_sb[:, i, :], a2a_out[b][i])
                for oc in range(CC):
                    psb = psA.tile([128, 2 * MT], fp32, tag="big", name="pp")
                    ps = psb[:, 0:MBLK]
                    for cc in range(CC):
                        nc.tensor.matmul(
                            ps[:],
                            lhsT=wp_sb[:, cc, oc * 128:(oc + 1) * 128],
                            rhs=y_sb[:, cc, :],
                            start=(cc == 0),
                            stop=(cc == CC - 1),
                        )
                    osb = ppool.tile([128, MBLK], fp32, tag="outsb", name="osb")
                    nc.scalar.activation(
                        osb[:], ps[:], AF.Identity, bias=bias_sb[:, oc:oc + 1]
                    )
                    nc.sync.dma_start(out[b, oc * 128:(oc + 1) * 128, :], osb[:])

            # ================= emission schedule =================
            # batch-0 QKV, chunk-streamed
            for cc in range(CC):
                load_chunk("r", 0, cc)
            for cc in range(CC):
                load_chunk("t", 0, cc)
            kt_half(0, 0, wk_sb, kT[0], "r")
            v_half(0, 0)
            kt_half(0, 1, wk_sb, kT[0], "r")
            v_half(0, 1)
            kt_half(0, 0, wq_sb, qT[0], "t")
            kt_half(0, 1, wq_sb, qT[0], "t")

            # batch-1 chunk loads queued early (DMA runs ahead of compute)
            for cc in range(CC):
                load_chunk("r", 1, cc)
            for cc in range(CC):
                load_chunk("t", 1, cc)

            # wproj/bias loads (needed by proj(0) mid-way through)
            wp_sb = wpool.tile([128, CC, C], bf16, name="wp_sb")
            for cc in range(CC):
                nc.sync.dma_start(wp_sb[:, cc, :], wproj[cc * 128:(cc + 1) * 128, :])
            bias_sb = wpool.tile([128, CC], fp32, name="bias_sb")
            nc.sync.dma_start(bias_sb[:], bproj.ap().rearrange("(a p) -> p a", p=128))

            # batch-1 QKV pieces interleaved between batch-0 attention m-tiles
            qkv1 = [
                lambda: kt_half(1, 0, wk_sb, kT[1], "r"),
                lambda: v_half(1, 0),
                lambda: (kt_half(1, 1, wk_sb, kT[1], "r"), v_half(1, 1)),
                lambda: (
                    kt_half(1, 0, wq_sb, qT[1], "t"),
                    kt_half(1, 1, wq_sb, qT[1], "t"),
                ),
            ]
            for mt in range(NMT):
                attn_mt(0, mt)
                qkv1[mt]()

            # batch-0 normalize + AllToAll fire while batch-1 attention runs
            norm_and_a2a(0)
            attn_mt(1, 0)
            attn_mt(1, 1)
            attn_mt(1, 2)
            proj(0)  # a2a(0) long done; PE work overlaps the tail AllToAll
            attn_mt(1, 3)
            norm_and_a2a(1)
            proj(1)

    nc.compile()
    return nc


def _shard_inputs(reference_data, target_data, Wq, Wkv, Wproj, bproj):
    import ml_dtypes

    bf16 = ml_dtypes.bfloat16
    xrefT = np.ascontiguousarray(
        np.asarray(reference_data, dtype=np.float32).transpose(0, 2, 1)
    ).astype(bf16)
    xtgtT = np.ascontiguousarray(
        np.asarray(target_data, dtype=np.float32).transpose(0, 2, 1)
    ).astype(bf16)
    Wq = np.asarray(Wq, dtype=np.float32)
    Wkv = np.asarray(Wkv, dtype=np.float32)
    Wproj_b = np.asarray(Wproj, dtype=np.float32).astype(bf16)
    bproj = np.asarray(bproj, dtype=np.float32)

    in_maps = []
    for c in range(NCORES):
        lo, hi = c * CHPC, (c + 1) * CHPC
        in_maps.append(
            {
                "xrefT": xrefT,
                "xtgtT": xtgtT,
                "wq": Wq[:, lo:hi].astype(bf16),
                "wk": Wkv[:, lo:hi].astype(bf16),
                "wv": Wkv[:, C + lo:C + hi].astype(bf16),
                "wproj": Wproj_b,
                "bproj": bproj,
            }
        )
    return in_maps


def _ensure_ntff_hook():
    """Register the axon NTFF profile hook if the image's antenv lacks it."""
    try:
        import antenv.axon_hooks  # noqa: F401

        return
    except ImportError:
        pass
    import sys
    import types

    import antenv

    mod = types.ModuleType("antenv.axon_hooks")
    state = {"hook": None}
    mod.set_axon_ntff_profile_hook = lambda h: state.__setitem__("hook", h)
    mod.get_axon_ntff_profile_hook = lambda: state["hook"]
    sys.modules["antenv.axon_hooks"] = mod
    antenv.axon_hooks = mod
    try:
        from trn_agent_boot.trn_boot import _ntff_profile_via_ctypes

        mod.set_axon_ntff_profile_hook(
            _ntff_profile_via_ctypes("/opt/axon/libaxon_pjrt.so")
        )
    except Exception:
        pass


def run(inputs: dict, trace: bool = False):
    """Compile (cached), run on 8 cores, return (full_output, BassKernelResults)."""
    from concourse.bass_utils import run_bass_kernel_spmd

    if trace:
        _ensure_ntff_hook()
    nc = _build()
    in_maps = _shard_inputs(**inputs)
    res = run_bass_kernel_spmd(
        nc, in_maps, core_ids=list(range(NCORES)), trace=trace
    )
    full = np.zeros((B, M, C), dtype=np.float32)
    for c in range(NCORES):
        blk = np.asarray(res.results[c]["out"], dtype=np.float32)  # [B, C, MBLK]
        for b in range(B):
            full[b, c * MBLK:(c + 1) * MBLK, :] = blk[b].T
    return full, res


def kernel(reference_data, target_data, Wq, Wkv, Wproj, bproj) -> np.ndarray:
    full, _ = run(
        {
            "reference_data": reference_data,
            "target_data": target_data,
            "Wq": Wq,
            "Wkv": Wkv,
            "Wproj": Wproj,
            "bproj": bproj,
        }
    )
    return full
